# revision 11
# baseline (speedup 1.0000x reference)
"""FitzHugh-Nagumo Euler recurrence kernel for Trainium2 (8 NeuronCores).

Problem: z (16, 2000, 1024) f32 -> V (16, 2000, 1024) f32 where
    v[k+1] = (1+dt) v[k] - (dt/3) v[k]^3 - dt s[k] + dt z[k]
    s[k+1] = s[k] + dt*0.08*(v[k] + 0.7 - 0.8 s[k])
    out[:, k] = v[k] / 2,  v[0] = s[0] = 0.

Strategy:
  - Pure data parallel over the B*L = 16384 independent lanes: 2 batches
    per core x 8 cores -> 2048 lanes/core = 128 partitions x 16 free.
  - First-order rewrite in u = v/2 with a combined slow state
    R~ = (W - K2 - zd)/(dt*beta), W = (dt/2)s, which absorbs both the
    recovery variable s and the z-forcing, so each Euler step is exactly
    3 fused DVE instructions (M, U, R) whose producer->consumer
    distances are all >= 2 (avoids the DVE read-write turnaround bubble
    on back-to-back dependent ops):
        S~[k]  = 62.5*((alpha z[k]) - z[k+1]) + 0.35      (bulk: ACT
                 scale-only copies + Pool tensor ops - off the DVE)
        opM[k]: m~[k]  = u[k] + S~[k]                     (DVE tensor_tensor)
        opU[k]: u[k+1] = (1.1 u[k] - dtb R~[k]) + u^2 (u * -4cg)
                                                          (custom DVE op)
        opR[k]: R~[k+1] = alpha R~[k] + m~[k]             (DVE stt)
    u_0 = 0, R~_0 = -62.5 z_0. Exactly equivalent to the reference
    recurrence; carries the slow s-state explicitly so fp32 roundoff is
    not amplified (an s-eliminating second-order form loses ~100x
    accuracy). 3 DVE instructions per Euler step is the floor: the R~
    update has 3 independent tensor inputs and DVE ops take only 2.
    Every engine op used is bit-exact fp32, and the op-level association
    was chosen (validated bit-exactly against a numpy replica) to
    minimize the amplified end-to-end divergence from the fp32 jax
    reference.

  Performance (TimelineSim cost model = the graded metric):
  - A same-engine producer->consumer data dependency costs ~177.4ns
    (processing 77 + write-ack 60 + completion-sem 28+7+eps) before the
    consumer may start; ops at emission distance >= 3 are free of it.
    The m->R->u coupling forms an irreducible 3-edge cycle per 2 Euler
    steps (any exact 2-input-op formulation has >= 3 tensor merges per
    u->R->u round trip), so the per-step floor is 3*177.4/2 = 266ns.
    Measured dead ends: per-step Pool/ACT handoffs (launch+sem round
    trip 230-420ns), lane-splitting (op cost is init-dominated, count
    doubles), the s-eliminated 2nd-order form (passes accuracy at
    3.8e-3 but its unavoidable adjacent dependency serializes to
    ~283ns/step on the in-order engine), PE/PSUM accumulators (DVE
    PSUM-operand access init poisons the u-op), and tensor_tensor_scan
    blocks (one recurrence per partition; causality forbids batching
    the u-coupled R-scan).
  - Perfetto-level trace analysis (fake-perfetto recorder against
    TimelineSim) found the above-floor overheads: (1) each S~ prep
    GROUP costs a fixed ~117ns coalesced-semaphore epoch crossing at
    the first DVE consumer - so steady chunks use ONE prep range, not
    halves; (2) the r0-init ACT op must be emitted before the bulk
    prep ACT ops (in-order engine) or the first opU stalls ~3us;
    (3) the first chunk's earliest S~ slices are computed ON the DVE
    (same fp32 association, bit-identical) to skip the ACT->Pool
    cross-engine warmup latency; (4) chunk size 100 with half-chunk
    output flushes (quarter flushes for the last chunk) minimizes
    ramp + drain. 525575ns vs 535284 baseline.
"""

import numpy as np

DT = 0.1
B, T, L = 16, 2000, 1024
NCORES = 8
P = 128                      # SBUF partitions
FD = (2 * L) // P            # 16 lanes per partition per core
TC = 100                     # timesteps per chunk
NCH = T // TC                # 20 chunks

ALPHA = 1.0 - DT * 0.08 * 0.8          # s-decay       0.9936
BETA = DT * 0.08                       # s<-v coupling 0.008
GAMMA = DT * 0.08 * 0.7                # s const       0.0056
CG = DT / 3.0
DTB = DT * BETA              # 0.0008

# u = v/2 scaled coefficients (first-order scheme)
OPA_C0 = 1.0 + DT            # * u_k
OPA_C1 = -4.0 * CG           # * u_k^3
OPA_C2 = -DTB                # * R~_k
INV2B = 1.0 / (2.0 * BETA)   # 62.5
SB = GAMMA / (2.0 * BETA)    # 0.35 (S~ bias)

# first-chunk staging (see docstring, perf item 3/4)
FIRST_Z = (6, 14, 42, 62, 101)       # z-DMA piece boundaries (cols)
FIRST_S = (0, 4, 12, 24, 40, 60)     # S~ prep slice boundaries (cols)
DVE_PREP_SLICES = 2                  # first N slices prepped on the DVE
OUT_SPLIT = 2                        # output flushes per chunk
LAST_FLUSH_SPLIT = 10                # finer flushes for the last chunk (drain)

_CACHE = {}


def _register_ops():
    """Runtime-register the fused FHN custom DVE step op."""
    from concourse.dve_spec import (
        Spec, Src0, Src1, C0 as C0L, C1 as C1L, C2 as C2L, sq, lower,
    )
    from concourse.dve_uop import DveOpSpec
    import concourse.dve_ops as dve_ops
    from concourse.dve_ops import DveOp, OPS

    def make_op(name, spec):
        if name in dve_ops._SUB_OPCODE_FOR_NAME:
            for op in OPS:
                if op.name == name:
                    return op
        row = 1 + len(OPS)
        assert row < 0x20
        shas = {}
        for ver in ("v3", "v4"):
            s = DveOpSpec(name=name, opcode=row, uops=lower(spec, ver=ver), rd1_en=True)
            shas[ver] = s.sha(ver)
        op = DveOp(name, spec, subdim=False, uops_sha=shas)
        OPS.append(op)
        dve_ops._SUB_OPCODE_FOR_NAME[name] = row
        dve_ops.CUSTOM_DVE_SPECS[name] = spec
        return op

    # u' = (u*C0 + R*C2) + u^2*(u*C1) — this exact fp32 association is
    # load-bearing: the recurrence amplifies per-step rounding ~1e3x over
    # T=2000 steps, and this tree gives the smallest end-to-end deviation
    # from the fp32 jax reference among the equivalent associations.
    fh_u = make_op(
        "FH_U7_ANT",
        Spec(
            body=(Src0 * C0L + Src1 * C2L) + sq(Src0) * (Src0 * C1L),
            reference=lambda in0, in1, s0, s1, imm2: (
                (in0 * s0 + in1 * imm2) + (in0 * in0) * (in0 * np.float32(s1))
            ).astype(np.float32),
        ),
    )
    return fh_u


def _build_program():
    import concourse.bacc as bacc
    import concourse.mybir as mybir
    from concourse.tile import TileContext

    fh_u = _register_ops()
    f32 = mybir.dt.float32
    au = mybir.AluOpType

    nc = bacc.Bacc("TRN2", target_bir_lowering=False, debug=False)
    z_d = nc.dram_tensor("z", [P, T * FD], f32, kind="ExternalInput")
    v_d = nc.dram_tensor("v", [P, T * FD], f32, kind="ExternalOutput")
    z_ap = z_d.ap()
    v_ap = v_d.ap()

    Copy = mybir.ActivationFunctionType.Copy
    with TileContext(nc) as tc:
        with (
            tc.tile_pool(name="zp", bufs=3) as zp,
            tc.tile_pool(name="wp", bufs=2) as wp,
            tc.tile_pool(name="stp", bufs=4) as stp,
            tc.tile_pool(name="vp", bufs=3) as vp,
            tc.tile_pool(name="small", bufs=1) as sp,
        ):
            r_t = [
                sp.tile([P, FD], f32, tag="r0", name="r0"),
                sp.tile([P, FD], f32, tag="r1", name="r1"),
            ]
            m_t = [
                sp.tile([P, FD], f32, tag="m0", name="m0"),
                sp.tile([P, FD], f32, tag="m1", name="m1"),
            ]
            # 0.35-filled const tile for the S~ bias (ACT's fused
            # scale+bias Copy is NOT bit-exact fp32; scale-only is, so the
            # bias is added separately on the Pool engine). Filled on the
            # DVE: it runs in the otherwise-dead z-DMA wait at startup and
            # removes a cross-engine wait from the first DVE-prep op.
            c35_t = sp.tile([P, TC * FD], f32, tag="c35", name="c35")
            z_tiles, s_tiles, v_tiles = {}, {}, {}

            def prep_pool(zt, st, wt, lo, hi):
                """S~ = (62.5*((alpha*z_k) - z_{k+1})) + 0.35, exact fp32 in
                this association (ACT scale-only copies and Pool adds are
                bit-exact; no op writes a tile it reads)."""
                a, b = lo * FD, hi * FD
                nc.scalar.activation(wt[:, a:b], zt[:, a:b], Copy, scale=float(ALPHA))
                nc.gpsimd.tensor_tensor(
                    out=st[:, a:b], in0=wt[:, a:b],
                    in1=zt[:, a + FD : b + FD], op=au.subtract)
                nc.scalar.activation(wt[:, a:b], st[:, a:b], Copy, scale=float(INV2B))
                nc.gpsimd.tensor_tensor(
                    out=st[:, a:b], in0=wt[:, a:b], in1=c35_t[:, a:b], op=au.add)

            def prep_dve(zt, st, wt, lo, hi):
                """Same fp32 association on the DVE (startup slices only):
                w = fl(fl(z*alpha) - z1); S~ = fl(fl(w*62.5) + 0.35)."""
                a, b = lo * FD, hi * FD
                nc.vector.scalar_tensor_tensor(
                    wt[:, a:b], zt[:, a:b], ALPHA, zt[:, a + FD : b + FD],
                    op0=au.mult, op1=au.subtract)
                nc.vector.scalar_tensor_tensor(
                    st[:, a:b], wt[:, a:b], INV2B, c35_t[:, a:b],
                    op0=au.mult, op1=au.add)

            def fetch_chunk(c):
                """Emit Z-DMA + bulk S~ prep (ACT + Pool, off the DVE)."""
                if c in z_tiles or c >= NCH:
                    return
                zcols = min(TC + 1, T - c * TC)
                zt = zp.tile([P, zcols * FD], f32, tag="z", name="zt")
                if c == 0:
                    # fine-grained first chunk so the step loop starts early
                    lo = 0
                    for hi in FIRST_Z:
                        hi = min(hi, zcols)
                        if hi > lo:
                            nc.sync.dma_start(
                                zt[:, lo * FD : hi * FD],
                                z_ap[:, lo * FD : hi * FD])
                        lo = hi
                    # r0-init before the bulk prep ACT ops (in-order engine)
                    nc.scalar.activation(
                        r_t[0][:], zt[:, 0:FD], Copy, scale=float(-INV2B))
                else:
                    nc.sync.dma_start(
                        zt[:], z_ap[:, c * TC * FD : (c * TC + zcols) * FD])
                z_tiles[c] = zt
                m = min(TC, (T - 2) - c * TC)  # S~ entries in this chunk
                wt = wp.tile([P, TC * FD], f32, tag="w", name="wt")
                st = stp.tile([P, TC * FD], f32, tag="s", name="st")
                if c == 0:
                    qs = sorted(set([q for q in FIRST_S if q < m] + [m]))
                else:
                    # single range per chunk: each prep group costs one fixed
                    # ~117ns sem-epoch crossing at its first DVE consumer
                    qs = (0, m)
                for idx, (lo, hi) in enumerate(zip(qs[:-1], qs[1:])):
                    if hi <= lo:
                        continue
                    if c == 0 and idx < DVE_PREP_SLICES:
                        prep_dve(zt, st, wt, lo, hi)
                    else:
                        prep_pool(zt, st, wt, lo, hi)
                s_tiles[c] = st

            def v_tile(c):
                if c not in v_tiles:
                    v_tiles[c] = vp.tile([P, TC * FD], f32, tag="v", name="vt")
                return v_tiles[c]

            def vcol(k):
                return v_tile(k // TC)[:, (k % TC) * FD : (k % TC + 1) * FD]

            # u_0 = 0 and the c35 const fill go first on the DVE (they fill
            # the z-DMA dead time); then chunk 0 + prefetch chunk 1
            nc.vector.memset(vcol(0), 0.0)
            nc.vector.memset(c35_t[:], SB)
            fetch_chunk(0)
            fetch_chunk(1)

            for k in range(T - 1):
                cj, oj = k // TC, k % TC
                if oj == TC // 2:
                    fetch_chunk(cj + 2)  # prefetch (1.5 chunks of lead)
                if k <= T - 3:
                    # m~ = u_k + S~_k
                    nc.vector.tensor_tensor(
                        out=m_t[k % 2][:], in0=vcol(k),
                        in1=s_tiles[cj][:, oj * FD : (oj + 1) * FD], op=au.add,
                    )
                # u' = 1.1 u - 4cg u^3 - dtb R~
                nc.vector._custom_dve(
                    fh_u, out=vcol(k + 1), in0=vcol(k), in1=r_t[k % 2][:],
                    s0=OPA_C0, s1=OPA_C1, imm2=OPA_C2,
                )
                if k <= T - 3:
                    # R~' = alpha R~ + m~
                    nc.vector.scalar_tensor_tensor(
                        r_t[(k + 1) % 2][:], r_t[k % 2][:], ALPHA, m_t[k % 2][:],
                        op0=au.mult, op1=au.add,
                    )
                cc = (k + 1) // TC
                if cc < NCH - 1:
                    piece = TC // OUT_SPLIT
                    if (k + 1) % piece == piece - 1:
                        j = ((k + 1) % TC) // piece
                        lo = (cc * TC + j * piece) * FD
                        nc.sync.dma_start(
                            v_ap[:, lo : lo + piece * FD],
                            v_tiles[cc][:, j * piece * FD : (j + 1) * piece * FD],
                        )
                else:
                    # last chunk: fine pieces, then all-but-one column at
                    # T-2 and a single column after the final opU so the
                    # end-of-program drain waits only a minimal DMA.
                    piece = TC // LAST_FLUSH_SPLIT
                    if (k + 1) % piece == piece - 1 and (k + 1) <= T - piece - 1:
                        j = ((k + 1) % TC) // piece
                        lo = (cc * TC + j * piece) * FD
                        nc.sync.dma_start(
                            v_ap[:, lo : lo + piece * FD],
                            v_tiles[cc][:, j * piece * FD : (j + 1) * piece * FD],
                        )
                    elif (k + 1) == T - 2:
                        lo0 = (T - piece) * FD
                        nc.sync.dma_start(
                            v_ap[:, lo0 : (T - 1) * FD],
                            v_tiles[cc][:, (TC - piece) * FD : (TC - 1) * FD],
                        )
                    elif (k + 1) == T - 1:
                        nc.sync.dma_start(
                            v_ap[:, (T - 1) * FD :],
                            v_tiles[cc][:, (TC - 1) * FD :],
                        )
    nc.compile()
    return nc


def _get_program():
    if "nc" not in _CACHE:
        _CACHE["nc"] = _build_program()
    return _CACHE["nc"]


def _shard_input(z):
    """z (B,T,L) -> list of 8 per-core arrays (P, T*FD), lane-major layout."""
    shards = []
    for c in range(NCORES):
        zc = z[2 * c : 2 * c + 2]                      # (2, T, L)
        arr = zc.transpose(0, 2, 1).reshape(2 * L, T)  # (lane, T)
        arr = arr.reshape(P, FD, T).transpose(0, 2, 1) # (P, T, FD)
        shards.append(np.ascontiguousarray(arr, dtype=np.float32).reshape(P, T * FD))
    return shards


def _unshard_output(outs):
    """list of 8 (P, T*FD) -> (B, T, L)."""
    full = np.empty((B, T, L), dtype=np.float32)
    for c, o in enumerate(outs):
        arr = o.reshape(P, T, FD).transpose(0, 2, 1).reshape(2 * L, T)
        full[2 * c : 2 * c + 2] = arr.reshape(2, L, T).transpose(0, 2, 1)
    return full


def kernel(z, _trace=False):
    import time

    from concourse.bass_utils import run_bass_kernel_spmd

    z = np.asarray(z, dtype=np.float32)
    assert z.shape == (B, T, L), z.shape
    nc = _get_program()
    in_maps = [{"z": s} for s in _shard_input(z)]
    # The first execution of a freshly (re)loaded NEFF occasionally hits a
    # transient NRT_EXEC_UNIT_UNRECOVERABLE on the device (per-NEFF DVE
    # table reload); the device resets and the retry runs cleanly.
    last_exc = None
    for attempt in range(3):
        try:
            res = run_bass_kernel_spmd(
                nc, in_maps, core_ids=list(range(NCORES)), trace=_trace
            )
            break
        except Exception as e:  # noqa: BLE001 - retry transient device faults
            last_exc = e
            time.sleep(2.0 * (attempt + 1))
    else:
        raise last_exc
    out = _unshard_output([r["v"] for r in res.results])
    if _trace:
        _CACHE["last_results"] = res
    return out


# revision 12
# speedup vs baseline: 1.0007x; 1.0007x over previous
"""FitzHugh-Nagumo Euler recurrence kernel for Trainium2 (8 NeuronCores).

Problem: z (16, 2000, 1024) f32 -> V (16, 2000, 1024) f32 where
    v[k+1] = (1+dt) v[k] - (dt/3) v[k]^3 - dt s[k] + dt z[k]
    s[k+1] = s[k] + dt*0.08*(v[k] + 0.7 - 0.8 s[k])
    out[:, k] = v[k] / 2,  v[0] = s[0] = 0.

Strategy:
  - Pure data parallel over the B*L = 16384 independent lanes: 2 batches
    per core x 8 cores -> 2048 lanes/core = 128 partitions x 16 free.
  - First-order rewrite in u = v/2 with a combined slow state
    R~ = (W - K2 - zd)/(dt*beta), W = (dt/2)s, which absorbs both the
    recovery variable s and the z-forcing, so each Euler step is exactly
    3 fused DVE instructions (M, U, R) whose producer->consumer
    distances are all >= 2 (avoids the DVE read-write turnaround bubble
    on back-to-back dependent ops):
        S~[k]  = 62.5*((alpha z[k]) - z[k+1]) + 0.35      (bulk: ACT
                 scale-only copies + Pool tensor ops - off the DVE)
        opM[k]: m~[k]  = u[k] + S~[k]                     (DVE tensor_tensor)
        opU[k]: u[k+1] = (1.1 u[k] - dtb R~[k]) + u^2 (u * -4cg)
                                                          (custom DVE op)
        opR[k]: R~[k+1] = alpha R~[k] + m~[k]             (DVE stt)
    u_0 = 0, R~_0 = -62.5 z_0. Exactly equivalent to the reference
    recurrence; carries the slow s-state explicitly so fp32 roundoff is
    not amplified (an s-eliminating second-order form loses ~100x
    accuracy). 3 DVE instructions per Euler step is the floor: the R~
    update has 3 independent tensor inputs and DVE ops take only 2.
    Every engine op used is bit-exact fp32, and the op-level association
    was chosen (validated bit-exactly against a numpy replica) to
    minimize the amplified end-to-end divergence from the fp32 jax
    reference.

  Performance (TimelineSim cost model = the graded metric):
  - A same-engine producer->consumer data dependency costs ~177.4ns
    (processing 77 + write-ack 60 + completion-sem 28+7+eps) before the
    consumer may start; ops at emission distance >= 3 are free of it.
    The m->R->u coupling forms an irreducible 3-edge cycle per 2 Euler
    steps (any exact 2-input-op formulation has >= 3 tensor merges per
    u->R->u round trip), so the per-step floor is 3*177.4/2 = 266ns.
    Measured dead ends: per-step Pool/ACT handoffs (launch+sem round
    trip 230-420ns), lane-splitting (op cost is init-dominated, count
    doubles), the s-eliminated 2nd-order form (passes accuracy at
    3.8e-3 but its unavoidable adjacent dependency serializes to
    ~283ns/step on the in-order engine), PE/PSUM accumulators (DVE
    PSUM-operand access init poisons the u-op), and tensor_tensor_scan
    blocks (one recurrence per partition; causality forbids batching
    the u-coupled R-scan).
  - Perfetto-level trace analysis (fake-perfetto recorder against
    TimelineSim) found the above-floor overheads: (1) each S~ prep
    GROUP costs a fixed ~117ns coalesced-semaphore epoch crossing at
    the first DVE consumer - so steady chunks use ONE prep range, not
    halves; (2) the r0-init ACT op must be emitted before the bulk
    prep ACT ops (in-order engine) or the first opU stalls ~3us;
    (3) the first chunk's earliest S~ slices are computed ON the DVE
    (same fp32 association, bit-identical) to skip the ACT->Pool
    cross-engine warmup latency; (4) chunk size 100 with half-chunk
    output flushes (quarter flushes for the last chunk) minimizes
    ramp + drain. 525575ns vs 535284 baseline.
"""

import numpy as np

DT = 0.1
B, T, L = 16, 2000, 1024
NCORES = 8
P = 128                      # SBUF partitions
FD = (2 * L) // P            # 16 lanes per partition per core
TC = 100                     # timesteps per chunk
NCH = T // TC                # 20 chunks

ALPHA = 1.0 - DT * 0.08 * 0.8          # s-decay       0.9936
BETA = DT * 0.08                       # s<-v coupling 0.008
GAMMA = DT * 0.08 * 0.7                # s const       0.0056
CG = DT / 3.0
DTB = DT * BETA              # 0.0008

# u = v/2 scaled coefficients (first-order scheme)
OPA_C0 = 1.0 + DT            # * u_k
OPA_C1 = -4.0 * CG           # * u_k^3
OPA_C2 = -DTB                # * R~_k
INV2B = 1.0 / (2.0 * BETA)   # 62.5
SB = GAMMA / (2.0 * BETA)    # 0.35 (S~ bias)

# first-chunk staging (see docstring, perf item 3/4)
FIRST_Z = (6, 14, 42, 62, 101)       # z-DMA piece boundaries (cols)
FIRST_S = (0, 4, 12, 24, 40, 60)     # S~ prep slice boundaries (cols)
DVE_PREP_SLICES = 2                  # first N slices prepped on the DVE
OUT_SPLIT = 2                        # output flushes per chunk
LAST_FLUSH_SPLIT = 10                # finer flushes for the last chunk (drain)

_CACHE = {}


def _register_ops():
    """Runtime-register the fused FHN custom DVE step op."""
    from concourse.dve_spec import (
        Spec, Src0, Src1, C0 as C0L, C1 as C1L, C2 as C2L, sq, lower,
    )
    from concourse.dve_uop import DveOpSpec
    import concourse.dve_ops as dve_ops
    from concourse.dve_ops import DveOp, OPS

    def make_op(name, spec):
        if name in dve_ops._SUB_OPCODE_FOR_NAME:
            for op in OPS:
                if op.name == name:
                    return op
        row = 1 + len(OPS)
        assert row < 0x20
        shas = {}
        for ver in ("v3", "v4"):
            s = DveOpSpec(name=name, opcode=row, uops=lower(spec, ver=ver), rd1_en=True)
            shas[ver] = s.sha(ver)
        op = DveOp(name, spec, subdim=False, uops_sha=shas)
        OPS.append(op)
        dve_ops._SUB_OPCODE_FOR_NAME[name] = row
        dve_ops.CUSTOM_DVE_SPECS[name] = spec
        return op

    # u' = (u*C0 + R*C2) + u^2*(u*C1) — this exact fp32 association is
    # load-bearing: the recurrence amplifies per-step rounding ~1e3x over
    # T=2000 steps, and this tree gives the smallest end-to-end deviation
    # from the fp32 jax reference among the equivalent associations.
    fh_u = make_op(
        "FH_U7_ANT",
        Spec(
            body=(Src0 * C0L + Src1 * C2L) + sq(Src0) * (Src0 * C1L),
            reference=lambda in0, in1, s0, s1, imm2: (
                (in0 * s0 + in1 * imm2) + (in0 * in0) * (in0 * np.float32(s1))
            ).astype(np.float32),
        ),
    )
    return fh_u


def _build_program():
    import concourse.bacc as bacc
    import concourse.mybir as mybir
    from concourse.tile import TileContext

    fh_u = _register_ops()
    f32 = mybir.dt.float32
    au = mybir.AluOpType

    nc = bacc.Bacc("TRN2", target_bir_lowering=False, debug=False)
    z_d = nc.dram_tensor("z", [P, T * FD], f32, kind="ExternalInput")
    v_d = nc.dram_tensor("v", [P, T * FD], f32, kind="ExternalOutput")
    z_ap = z_d.ap()
    v_ap = v_d.ap()

    Copy = mybir.ActivationFunctionType.Copy
    with TileContext(nc) as tc:
        with (
            tc.tile_pool(name="zp", bufs=3) as zp,
            tc.tile_pool(name="wp", bufs=2) as wp,
            tc.tile_pool(name="stp", bufs=4) as stp,
            tc.tile_pool(name="vp", bufs=3) as vp,
            tc.tile_pool(name="small", bufs=1) as sp,
        ):
            r_t = [
                sp.tile([P, FD], f32, tag="r0", name="r0"),
                sp.tile([P, FD], f32, tag="r1", name="r1"),
            ]
            m_t = [
                sp.tile([P, FD], f32, tag="m0", name="m0"),
                sp.tile([P, FD], f32, tag="m1", name="m1"),
            ]
            # 0.35-filled const tile for the S~ bias (ACT's fused
            # scale+bias Copy is NOT bit-exact fp32; scale-only is, so the
            # bias is added separately on the Pool engine). Filled on the
            # DVE: it runs in the otherwise-dead z-DMA wait at startup and
            # removes a cross-engine wait from the first DVE-prep op.
            c35_t = sp.tile([P, TC * FD], f32, tag="c35", name="c35")
            z_tiles, s_tiles, v_tiles = {}, {}, {}

            def prep_pool(zt, st, wt, lo, hi):
                """S~ = (62.5*((alpha*z_k) - z_{k+1})) + 0.35, exact fp32 in
                this association (ACT scale-only copies and Pool adds are
                bit-exact; no op writes a tile it reads)."""
                a, b = lo * FD, hi * FD
                nc.scalar.activation(wt[:, a:b], zt[:, a:b], Copy, scale=float(ALPHA))
                nc.gpsimd.tensor_tensor(
                    out=st[:, a:b], in0=wt[:, a:b],
                    in1=zt[:, a + FD : b + FD], op=au.subtract)
                nc.scalar.activation(wt[:, a:b], st[:, a:b], Copy, scale=float(INV2B))
                nc.gpsimd.tensor_tensor(
                    out=st[:, a:b], in0=wt[:, a:b], in1=c35_t[:, a:b], op=au.add)

            def prep_dve(zt, st, wt, lo, hi):
                """Same fp32 association on the DVE (startup slices only):
                w = fl(fl(z*alpha) - z1); S~ = fl(fl(w*62.5) + 0.35)."""
                a, b = lo * FD, hi * FD
                nc.vector.scalar_tensor_tensor(
                    wt[:, a:b], zt[:, a:b], ALPHA, zt[:, a + FD : b + FD],
                    op0=au.mult, op1=au.subtract)
                nc.vector.scalar_tensor_tensor(
                    st[:, a:b], wt[:, a:b], INV2B, c35_t[:, a:b],
                    op0=au.mult, op1=au.add)

            def fetch_chunk(c):
                """Emit Z-DMA + bulk S~ prep (ACT + Pool, off the DVE)."""
                if c in z_tiles or c >= NCH:
                    return
                zcols = min(TC + 1, T - c * TC)
                zt = zp.tile([P, zcols * FD], f32, tag="z", name="zt")
                if c == 0:
                    # fine-grained first chunk so the step loop starts early
                    lo = 0
                    for hi in FIRST_Z:
                        hi = min(hi, zcols)
                        if hi > lo:
                            nc.sync.dma_start(
                                zt[:, lo * FD : hi * FD],
                                z_ap[:, lo * FD : hi * FD])
                        lo = hi
                    # r0-init before the bulk prep ACT ops (in-order engine)
                    nc.scalar.activation(
                        r_t[0][:], zt[:, 0:FD], Copy, scale=float(-INV2B))
                else:
                    nc.sync.dma_start(
                        zt[:], z_ap[:, c * TC * FD : (c * TC + zcols) * FD])
                z_tiles[c] = zt
                m = min(TC, (T - 2) - c * TC)  # S~ entries in this chunk
                wt = wp.tile([P, TC * FD], f32, tag="w", name="wt")
                st = stp.tile([P, TC * FD], f32, tag="s", name="st")
                if c == 0:
                    qs = sorted(set([q for q in FIRST_S if q < m] + [m]))
                else:
                    # single range per chunk: each prep group costs one fixed
                    # ~117ns sem-epoch crossing at its first DVE consumer
                    qs = (0, m)
                for idx, (lo, hi) in enumerate(zip(qs[:-1], qs[1:])):
                    if hi <= lo:
                        continue
                    if c == 0 and idx < DVE_PREP_SLICES:
                        prep_dve(zt, st, wt, lo, hi)
                    else:
                        prep_pool(zt, st, wt, lo, hi)
                s_tiles[c] = st

            def v_tile(c):
                if c not in v_tiles:
                    v_tiles[c] = vp.tile([P, TC * FD], f32, tag="v", name="vt")
                return v_tiles[c]

            def vcol(k):
                return v_tile(k // TC)[:, (k % TC) * FD : (k % TC + 1) * FD]

            # u_0 = 0 and the c35 const fill go first on the DVE (they fill
            # the z-DMA dead time); then chunk 0 + prefetch chunk 1
            nc.vector.memset(vcol(0), 0.0)
            nc.vector.memset(c35_t[:], SB)
            fetch_chunk(0)
            fetch_chunk(1)

            for k in range(T - 1):
                cj, oj = k // TC, k % TC
                if oj == TC // 2:
                    fetch_chunk(cj + 2)  # prefetch (1.5 chunks of lead)
                if k <= T - 3:
                    # m~ = u_k + S~_k
                    nc.vector.tensor_tensor(
                        out=m_t[k % 2][:], in0=vcol(k),
                        in1=s_tiles[cj][:, oj * FD : (oj + 1) * FD], op=au.add,
                    )
                # u' = 1.1 u - 4cg u^3 - dtb R~
                nc.vector._custom_dve(
                    fh_u, out=vcol(k + 1), in0=vcol(k), in1=r_t[k % 2][:],
                    s0=OPA_C0, s1=OPA_C1, imm2=OPA_C2,
                )
                if k <= T - 3:
                    # R~' = alpha R~ + m~
                    nc.vector.scalar_tensor_tensor(
                        r_t[(k + 1) % 2][:], r_t[k % 2][:], ALPHA, m_t[k % 2][:],
                        op0=au.mult, op1=au.add,
                    )
                cc = (k + 1) // TC
                split = LAST_FLUSH_SPLIT if cc == NCH - 1 else OUT_SPLIT
                piece = TC // split
                if (k + 1) % piece == piece - 1:
                    j = ((k + 1) % TC) // piece
                    lo = (cc * TC + j * piece) * FD
                    nc.sync.dma_start(
                        v_ap[:, lo : lo + piece * FD],
                        v_tiles[cc][:, j * piece * FD : (j + 1) * piece * FD],
                    )
    nc.compile()
    return nc


def _get_program():
    if "nc" not in _CACHE:
        _CACHE["nc"] = _build_program()
    return _CACHE["nc"]


def _shard_input(z):
    """z (B,T,L) -> list of 8 per-core arrays (P, T*FD), lane-major layout."""
    shards = []
    for c in range(NCORES):
        zc = z[2 * c : 2 * c + 2]                      # (2, T, L)
        arr = zc.transpose(0, 2, 1).reshape(2 * L, T)  # (lane, T)
        arr = arr.reshape(P, FD, T).transpose(0, 2, 1) # (P, T, FD)
        shards.append(np.ascontiguousarray(arr, dtype=np.float32).reshape(P, T * FD))
    return shards


def _unshard_output(outs):
    """list of 8 (P, T*FD) -> (B, T, L)."""
    full = np.empty((B, T, L), dtype=np.float32)
    for c, o in enumerate(outs):
        arr = o.reshape(P, T, FD).transpose(0, 2, 1).reshape(2 * L, T)
        full[2 * c : 2 * c + 2] = arr.reshape(2, L, T).transpose(0, 2, 1)
    return full


def kernel(z, _trace=False):
    import time

    from concourse.bass_utils import run_bass_kernel_spmd

    z = np.asarray(z, dtype=np.float32)
    assert z.shape == (B, T, L), z.shape
    nc = _get_program()
    in_maps = [{"z": s} for s in _shard_input(z)]
    # The first execution of a freshly (re)loaded NEFF occasionally hits a
    # transient NRT_EXEC_UNIT_UNRECOVERABLE on the device (per-NEFF DVE
    # table reload); the device resets and the retry runs cleanly.
    last_exc = None
    for attempt in range(3):
        try:
            res = run_bass_kernel_spmd(
                nc, in_maps, core_ids=list(range(NCORES)), trace=_trace
            )
            break
        except Exception as e:  # noqa: BLE001 - retry transient device faults
            last_exc = e
            time.sleep(2.0 * (attempt + 1))
    else:
        raise last_exc
    out = _unshard_output([r["v"] for r in res.results])
    if _trace:
        _CACHE["last_results"] = res
    return out
